# revision 1
# baseline (speedup 1.0000x reference)
"""CircleLoss (nn_CircleLoss_55482387529741) Trainium2 Bass kernel.

Math (B=8192, D=128, margin m=0.25, gamma=256=16^2):
  a = l2norm(A) rows, b = l2norm(B) rows, s_ij = a_i . b_j
  logit_neg = g*relu(s-m)*(s+m)  ==>  exp(logit_neg) = exp(max(16*s, 4)^2 - 16)
    (exact identity; cold entries s<=m give exactly exp(0)=1)
  lse_pos_i = logit_pos_ii = (w-12)(w-4) with w = min(16*s_ii, 12)  [in 16s units]
  loss_i = softplus(lse_pos_i + log(sum_{j!=i} exp(logit_neg_ij)))
  out = mean(loss)

Distribution: a-rows sharded 8 x 1024. Each core computes a [8192 x 1024]
"flipped" sim slab (partitions = b-rows, free = its a-rows) so that the
per-b-row 1/||B_j|| rides the custom DVE op's per-partition scalar.
B is rotated per core on host so each core's diagonal lands in local
b-blocks 0..7 (SPMD compile-time constant positions).

Engines: PE matmuls (fp32) + ones-matmul partition reduction of exp tiles;
one custom DVE pass z = sq(maxx(r*invb, 4)) per tile; one ACT pass
exp(z-16); GPSIMD does b-row sumsq + diagonal zeroing off the hot engines.
"""

import sys

for _p in ("/opt/trn_rl_repo",):
    if _p not in sys.path:
        sys.path.append(_p)

import numpy as np

import concourse.bass as bass
from concourse import bacc
import concourse.mybir as mybir
import concourse.tile as tile
from concourse.bass_utils import run_bass_kernel_spmd
from concourse.masks import make_identity

F32 = mybir.dt.float32
BF16 = mybir.dt.bfloat16
AF = mybir.ActivationFunctionType
OP = mybir.AluOpType

B = 8192
D = 128
NCORES = 8
MPC = B // NCORES  # 1024 a-rows per core
NB = B // 128  # 64 b-blocks
NA = MPC // 128  # 8 a-tiles
LN16 = float(np.log(16.0))

_cache = {}


def _get_custom_op():
    """Register (once) the fused clamp+square DVE op: out = sq(maxx(in0*s0, s1))."""
    from concourse import dve_ops
    from concourse.dve_spec import Spec, Src0, C0, C1, maxx, sq, lower
    from concourse.dve_spec import _has_src1 as has_src1
    from concourse.dve_uop import DveOpSpec

    name = "CIRCLE_CLAMP_SQ"
    for o in dve_ops.OPS:
        if o.name == name:
            return o

    def _ref(in0, in1, s0, s1, imm2):
        return np.square(
            np.maximum(in0.astype(np.float32) * np.float32(s0), np.float32(s1))
        ).astype(np.float32)

    spec = Spec(body=sq(maxx(Src0 * C0, C1)), reference=_ref)
    opcode = dve_ops._CUSTOM_DVE_ROW_BASE + len(dve_ops.OPS)
    assert opcode < 0x20
    shas = {}
    for ver in ("v3", "v4"):
        try:
            shas[ver] = DveOpSpec(
                name=name,
                opcode=opcode,
                uops=lower(spec, ver=ver),
                rd1_en=has_src1(spec),
            ).sha(ver)
        except Exception:
            pass
    op = dve_ops.DveOp(name, spec, subdim=False, uops_sha=shas)
    dve_ops.OPS.append(op)
    dve_ops.CUSTOM_DVE_SPECS[name] = spec
    dve_ops._SUB_OPCODE_FOR_NAME[name] = opcode
    return op


def _build():
    if "nc" in _cache:
        return _cache["nc"]
    op = _get_custom_op()
    nc = bacc.Bacc("TRN2", target_bir_lowering=False)

    a_in = nc.declare_dram_parameter("a_shard", [MPC, D], F32, isOutput=False)
    bT_in = nc.declare_dram_parameter("bT", [D, B], F32, isOutput=False)
    out = nc.declare_dram_parameter("losses", [MPC], F32, isOutput=True)
    S_scr = nc.dram_tensor("S_scratch", [MPC], F32)
    ssb_scr = nc.dram_tensor("ssb_scratch", [B], F32)
    out_pm = out.rearrange("(m p) -> p m", p=128)  # [128, 8] view

    with tile.TileContext(nc) as tc:
        with (
            tc.tile_pool(name="consts", bufs=1) as consts,
            tc.tile_pool(name="big", bufs=1) as big,
            tc.tile_pool(name="stage", bufs=4) as stage,
            tc.tile_pool(name="aprep", bufs=1) as aprep,
            tc.tile_pool(name="bsq", bufs=2) as bsqp,
            tc.tile_pool(name="zpool", bufs=3) as zpool,
            tc.tile_pool(name="epool", bufs=3) as epool,
            tc.tile_pool(name="stats", bufs=1) as stats,
            tc.tile_pool(name="psim", bufs=2, space="PSUM") as psim,
            tc.tile_pool(name="psacc", bufs=1, space="PSUM") as psacc,
            tc.tile_pool(name="ptr", bufs=2, space="PSUM") as ptr,
        ):
            # ---- constants ----
            eye = consts.tile([128, 128], F32, tag="eye")
            make_identity(nc, eye)
            antieye = consts.tile([128, 128], F32, tag="antieye")
            # antieye = 1 - eye
            nc.gpsimd.tensor_scalar(
                out=antieye, in0=eye, scalar1=-1.0, scalar2=1.0,
                op0=OP.mult, op1=OP.add,
            )
            ones = consts.tile([128, 1], BF16, tag="ones")
            nc.vector.memset(ones, 1.0)
            b_ln16 = consts.tile([128, 1], F32, tag="b_ln16")
            nc.vector.memset(b_ln16, LN16)
            b_m16 = consts.tile([128, 1], F32, tag="b_m16")
            nc.vector.memset(b_m16, -16.0)

            # ---- persistent tensors ----
            bT = big.tile([128, B], BF16, tag="bT")
            aT = big.tile([128, MPC], BF16, tag="aT")
            ssb = stats.tile([128, NB], F32, tag="ssb")
            invb = stats.tile([128, NB], F32, tag="invb")
            ssa = stats.tile([128, NA], F32, tag="ssa")
            inva16 = stats.tile([128, NA], F32, tag="inva16")
            rdiag = stats.tile([128, NA], F32, tag="rdiag")

            # ---- load bT (8 chunks) ----
            for k in range(8):
                nc.gpsimd.dma_start(
                    out=bT[:, k * 1024:(k + 1) * 1024],
                    in_=bT_in[:, k * 1024:(k + 1) * 1024],
                )

            # ---- a prep: sumsq -> inva16 -> scale -> transpose ----
            a_big = aprep.tile([128, NA, D], F32, tag="a_stage")
            nc.sync.dma_start(
                out=a_big, in_=a_in.rearrange("(i p) d -> p i d", p=128)
            )
            asq = aprep.tile([128, NA, D], F32, tag="a_sq")
            nc.gpsimd.tensor_mul(asq, a_big, a_big)
            nc.vector.tensor_reduce(
                out=ssa, in_=asq, axis=mybir.AxisListType.X, op=OP.add
            )
            lssa = stats.tile([128, NA], F32, tag="lssa")
            nc.scalar.activation(out=lssa, in_=ssa, func=AF.Ln)
            nc.scalar.activation(out=inva16, in_=lssa, func=AF.Exp, scale=-0.5, bias=b_ln16)
            a16 = aprep.tile([128, NA, D], F32, tag="a16")
            for i in range(NA):
                nc.vector.tensor_scalar(
                    out=a16[:, i, :], in0=a_big[:, i, :], scalar1=inva16[:, i:i + 1],
                    scalar2=None, op0=OP.mult,
                )
            for q in range(2):  # two psum batches of 4 transposes
                pt = ptr.tile([128, 512], F32, tag="atr")
                for j in range(4):
                    nc.tensor.transpose(
                        pt[:, j * 128:(j + 1) * 128], a16[:, q * 4 + j, :], eye
                    )
                nc.scalar.copy(out=aT[:, q * 512:(q + 1) * 512], in_=pt)

            # ---- b prep: sumsq from bT (gpsimd), reshape via dram ----
            lssb = stats.tile([128, NB], F32, tag="lssb")
            ssb_flat = stats.tile([1, B], F32, tag="ssb_flat")
            for g in range(8):
                bsq = bsqp.tile([128, 1024], F32, tag="b_sq")
                nc.gpsimd.tensor_mul(
                    bsq, bT[:, g * 1024:(g + 1) * 1024],
                    bT[:, g * 1024:(g + 1) * 1024],
                )
                nc.gpsimd.tensor_reduce(
                    out=ssb_flat[:, g * 1024:(g + 1) * 1024], in_=bsq,
                    axis=mybir.AxisListType.C, op=OP.add,
                )
                nc.sync.dma_start(
                    out=ssb_scr[g * 1024:(g + 1) * 1024],
                    in_=ssb_flat[0:1, g * 1024:(g + 1) * 1024],
                )
                nc.sync.dma_start(
                    out=ssb[:, g * 8:(g + 1) * 8],
                    in_=ssb_scr[g * 1024:(g + 1) * 1024].rearrange(
                        "(m p) -> p m", p=128
                    ),
                )
                nc.scalar.activation(
                    out=lssb[:, g * 8:(g + 1) * 8], in_=ssb[:, g * 8:(g + 1) * 8],
                    func=AF.Ln,
                )
                nc.scalar.activation(
                    out=invb[:, g * 8:(g + 1) * 8], in_=lssb[:, g * 8:(g + 1) * 8],
                    func=AF.Exp, scale=-0.5,
                )

            # ---- S accumulator psum [1, 1024] ----
            S_ps = psacc.tile([1, MPC], F32, tag="S")

            # ---- main loop over 64 b-blocks ----
            for m in range(NB):
                ps = psim.tile([128, MPC], F32, tag="sim")
                for h in range(2):
                    nc.tensor.matmul(
                        ps[:, h * 512:(h + 1) * 512],
                        bT[:, m * 128:(m + 1) * 128],
                        aT[:, h * 512:(h + 1) * 512],
                        start=True, stop=True,
                    )
                if m < NA:
                    dscr = stage.tile([128, 128], F32, tag="dscr")
                    nc.vector.tensor_mul(
                        dscr, ps[:, m * 128:(m + 1) * 128], eye
                    )
                    nc.vector.tensor_reduce(
                        out=rdiag[:, m:m + 1], in_=dscr,
                        axis=mybir.AxisListType.X, op=OP.add,
                    )
                z = zpool.tile([128, MPC], F32, tag="z")
                nc.vector._custom_dve(
                    op, out=z, in0=ps, s0=invb[:, m:m + 1], s1=4.0
                )
                e = epool.tile([128, MPC], BF16, tag="e")
                nc.scalar.activation(out=e, in_=z, func=AF.Exp, bias=b_m16)
                if m < NA:
                    nc.gpsimd.tensor_mul(
                        e[:, m * 128:(m + 1) * 128],
                        e[:, m * 128:(m + 1) * 128],
                        antieye,
                    )
                for h in range(2):
                    nc.tensor.matmul(
                        S_ps[:, h * 512:(h + 1) * 512],
                        ones,
                        e[:, h * 512:(h + 1) * 512],
                        start=(m == 0), stop=(m == NB - 1),
                        skip_group_check=True,
                    )

            # ---- epilogue: per-row losses ----
            S_sb = stats.tile([1, MPC], F32, tag="S_sb")
            nc.scalar.copy(out=S_sb, in_=S_ps)
            Srs = stats.tile([128, NA], F32, tag="Srs")
            nc.sync.dma_start(out=S_scr[:], in_=S_sb[0:1, :])
            nc.sync.dma_start(
                out=Srs, in_=S_scr.rearrange("(m p) -> p m", p=128)
            )
            lse = stats.tile([128, NA], F32, tag="lse")
            nc.scalar.activation(out=lse, in_=Srs, func=AF.Ln)
            sdiag = stats.tile([128, NA], F32, tag="sdiag")
            nc.vector.tensor_mul(sdiag, rdiag, invb[:, 0:NA])
            w = stats.tile([128, NA], F32, tag="w")
            nc.vector.tensor_scalar(
                out=w, in0=sdiag, scalar1=12.0, scalar2=None, op0=OP.min
            )
            lpr = stats.tile([128, NA], F32, tag="lpr")
            nc.vector.scalar_tensor_tensor(
                out=lpr, in0=w, scalar=16.0, in1=w, op0=OP.subtract, op1=OP.mult
            )
            t = stats.tile([128, NA], F32, tag="t")
            nc.vector.scalar_tensor_tensor(
                out=t, in0=lpr, scalar=48.0, in1=lse, op0=OP.add, op1=OP.add
            )
            abst = stats.tile([128, NA], F32, tag="abst")
            nc.scalar.activation(out=abst, in_=t, func=AF.Abs)
            u = stats.tile([128, NA], F32, tag="u")
            nc.scalar.activation(out=u, in_=abst, func=AF.Exp, scale=-1.0)
            v = stats.tile([128, NA], F32, tag="v")
            nc.scalar.activation(out=v, in_=u, func=AF.Ln, bias=1.0)
            loss = stats.tile([128, NA], F32, tag="loss")
            nc.vector.scalar_tensor_tensor(
                out=loss, in0=t, scalar=0.0, in1=v, op0=OP.max, op1=OP.add
            )
            nc.sync.dma_start(out=out_pm, in_=loss)

    nc.finalize()
    _cache["nc"] = nc
    return nc


def kernel(embeddings_a: np.ndarray, embeddings_b: np.ndarray) -> np.ndarray:
    nc = _build()
    A = np.ascontiguousarray(embeddings_a, dtype=np.float32)
    Bm = np.ascontiguousarray(embeddings_b, dtype=np.float32)
    in_maps = []
    for c in range(NCORES):
        br = np.roll(Bm, -MPC * c, axis=0)
        in_maps.append(
            {
                "a_shard": A[MPC * c:MPC * (c + 1)],
                "bT": np.ascontiguousarray(br.T),
            }
        )
    res = run_bass_kernel_spmd(nc, in_maps, list(range(NCORES))).results
    losses = np.concatenate([res[c]["losses"] for c in range(NCORES)])
    return np.float32(np.mean(losses.astype(np.float64)))



# revision 5
# speedup vs baseline: 7.2070x; 7.2070x over previous
"""CircleLoss (nn_CircleLoss_55482387529741) Trainium2 Bass kernel.

Math (B=8192, D=128, margin m=0.25, gamma=256=16^2):
  a = l2norm(A) rows, b = l2norm(B) rows, s_ij = a_i . b_j
  logit_neg = g*relu(s-m)*(s+m)  ==>  exp(logit_neg) = exp(max(16*s, 4)^2 - 16)
    (exact identity; cold entries s<=m give exactly exp(0)=1)
  lse_pos_i = logit_pos_ii = (w-12)(w-4) with w = min(16*s_ii, 12)  [in 16s units]
  loss_i = softplus(lse_pos_i + log(sum_{j!=i} exp(logit_neg_ij)))
  out = mean(loss)

Distribution: a-rows sharded 8 x 1024. Each core computes a [8192 x 1024]
"flipped" sim slab (partitions = b-rows, free = its a-rows) so that the
per-b-row 1/||B_j|| rides the custom DVE op's per-partition scalar.
B is rotated per core on host so each core's diagonal lands in local
b-blocks 0..7 (SPMD compile-time constant positions).

Engines: PE matmuls (fp32) + ones-matmul partition reduction of exp tiles;
one custom DVE pass z = sq(maxx(r*invb, 4)) per tile; one ACT pass
exp(z-16); GPSIMD does b-row sumsq + diagonal zeroing off the hot engines.
"""

import sys

for _p in ("/opt/trn_rl_repo",):
    if _p not in sys.path:
        sys.path.append(_p)

import numpy as np

import concourse.bass as bass
from concourse import bacc
import concourse.mybir as mybir
import concourse.tile as tile
from concourse.bass_utils import run_bass_kernel_spmd
from concourse.masks import make_identity

F32 = mybir.dt.float32
BF16 = mybir.dt.bfloat16
AF = mybir.ActivationFunctionType
OP = mybir.AluOpType

B = 8192
D = 128
NCORES = 8
MPC = B // NCORES  # 1024 a-rows per core
NB = B // 128  # 64 b-blocks
NA = MPC // 128  # 8 a-tiles
LN16 = float(np.log(16.0))

_cache = {}


def _get_custom_op():
    """Register (once) the fused clamp+square DVE op: out = sq(maxx(in0*s0, s1))."""
    from concourse import dve_ops
    from concourse.dve_spec import Spec, Src0, C0, C1, maxx, sq, lower
    from concourse.dve_spec import _has_src1 as has_src1
    from concourse.dve_uop import DveOpSpec

    name = "CIRCLE_CLAMP_SQ"
    for o in dve_ops.OPS:
        if o.name == name:
            return o

    def _ref(in0, in1, s0, s1, imm2):
        return np.square(
            np.maximum(in0.astype(np.float32) * np.float32(s0), np.float32(s1))
        ).astype(np.float32)

    spec = Spec(body=sq(maxx(Src0 * C0, C1)), reference=_ref)
    opcode = dve_ops._CUSTOM_DVE_ROW_BASE + len(dve_ops.OPS)
    assert opcode < 0x20
    shas = {}
    for ver in ("v3", "v4"):
        try:
            shas[ver] = DveOpSpec(
                name=name,
                opcode=opcode,
                uops=lower(spec, ver=ver),
                rd1_en=has_src1(spec),
            ).sha(ver)
        except Exception:
            pass
    op = dve_ops.DveOp(name, spec, subdim=False, uops_sha=shas)
    dve_ops.OPS.append(op)
    dve_ops.CUSTOM_DVE_SPECS[name] = spec
    dve_ops._SUB_OPCODE_FOR_NAME[name] = opcode
    return op


def _build():
    if "nc" in _cache:
        return _cache["nc"]
    op = _get_custom_op()
    nc = bacc.Bacc("TRN2", target_bir_lowering=False)

    a_in = nc.declare_dram_parameter("a_shard", [MPC, D], F32, isOutput=False)
    bT_in = nc.declare_dram_parameter("bT", [D, B], F32, isOutput=False)
    out = nc.declare_dram_parameter("losses", [MPC], F32, isOutput=True)
    S_scr = nc.dram_tensor("S_scratch", [MPC], F32)
    ssb_scr = nc.dram_tensor("ssb_scratch", [B], F32)
    out_pm = out.rearrange("(m p) -> p m", p=128)  # [128, 8] view

    with tile.TileContext(nc) as tc:
        with (
            tc.tile_pool(name="consts", bufs=1) as consts,
            tc.tile_pool(name="big", bufs=1) as big,
            tc.tile_pool(name="stage", bufs=4) as stage,
            tc.tile_pool(name="aprep", bufs=1) as aprep,
            tc.tile_pool(name="bsq", bufs=2) as bsqp,
            tc.tile_pool(name="zpool", bufs=3) as zpool,
            tc.tile_pool(name="epool", bufs=3) as epool,
            tc.tile_pool(name="stats", bufs=1) as stats,
            tc.tile_pool(name="psim", bufs=2, space="PSUM") as psim,
            tc.tile_pool(name="psacc", bufs=1, space="PSUM") as psacc,
            tc.tile_pool(name="ptr", bufs=1, space="PSUM") as ptr,
        ):
            # ---- constants ----
            eye = consts.tile([128, 128], F32, tag="eye")
            make_identity(nc, eye)
            antieye = consts.tile([128, 128], F32, tag="antieye")
            # antieye = 1 - eye
            nc.gpsimd.tensor_scalar(
                out=antieye, in0=eye, scalar1=-1.0, scalar2=1.0,
                op0=OP.mult, op1=OP.add,
            )
            ones = consts.tile([128, 1], BF16, tag="ones")
            nc.vector.memset(ones, 1.0)
            b_ln16 = consts.tile([128, 1], F32, tag="b_ln16")
            nc.vector.memset(b_ln16, LN16)
            b_m16 = consts.tile([128, 1], F32, tag="b_m16")
            nc.vector.memset(b_m16, -16.0)

            # ---- persistent tensors ----
            bT = big.tile([128, B], BF16, tag="bT")
            aT = big.tile([128, MPC], BF16, tag="aT")
            ssb = stats.tile([128, NB], F32, tag="ssb")
            invb = stats.tile([128, NB], F32, tag="invb")
            ssa = stats.tile([128, NA], F32, tag="ssa")
            inva16 = stats.tile([128, NA], F32, tag="inva16")
            rdiag = stats.tile([128, NA], F32, tag="rdiag")

            # ---- load bT (8 chunks) ----
            for k in range(8):
                nc.gpsimd.dma_start(
                    out=bT[:, k * 1024:(k + 1) * 1024],
                    in_=bT_in[:, k * 1024:(k + 1) * 1024],
                )

            # ---- a prep: sumsq -> inva16 -> scale -> transpose ----
            a_big = aprep.tile([128, NA, D], F32, tag="a_stage")
            nc.sync.dma_start(
                out=a_big, in_=a_in.rearrange("(i p) d -> p i d", p=128)
            )
            asq = aprep.tile([128, NA, D], F32, tag="a_sq")
            nc.gpsimd.tensor_mul(asq, a_big, a_big)
            nc.vector.tensor_reduce(
                out=ssa, in_=asq, axis=mybir.AxisListType.X, op=OP.add
            )
            lssa = stats.tile([128, NA], F32, tag="lssa")
            nc.scalar.activation(out=lssa, in_=ssa, func=AF.Ln)
            nc.scalar.activation(out=inva16, in_=lssa, func=AF.Exp, scale=-0.5, bias=b_ln16)
            a16 = aprep.tile([128, NA, D], F32, tag="a16")
            for i in range(NA):
                nc.vector.tensor_scalar(
                    out=a16[:, i, :], in0=a_big[:, i, :], scalar1=inva16[:, i:i + 1],
                    scalar2=None, op0=OP.mult,
                )
            for q in range(2):  # two psum batches of 4 transposes
                pt = ptr.tile([128, 512], F32, tag="atr")
                for j in range(4):
                    nc.tensor.transpose(
                        pt[:, j * 128:(j + 1) * 128], a16[:, q * 4 + j, :], eye
                    )
                nc.scalar.copy(out=aT[:, q * 512:(q + 1) * 512], in_=pt)

            # ---- b prep: sumsq via ones-matmul column sums (PE), reshape via dram ----
            lssb = stats.tile([128, NB], F32, tag="lssb")
            ssb_flat = stats.tile([1, B], F32, tag="ssb_flat")
            for g in range(8):
                bsq = bsqp.tile([128, 1024], BF16, tag="b_sq")
                nc.gpsimd.tensor_mul(
                    bsq, bT[:, g * 1024:(g + 1) * 1024],
                    bT[:, g * 1024:(g + 1) * 1024],
                )
                for h in range(2):
                    pssb = ptr.tile([1, 512], F32, tag="bsum")
                    nc.tensor.matmul(
                        pssb,
                        ones,
                        bsq[:, h * 512:(h + 1) * 512],
                        start=True, stop=True,
                    )
                    nc.scalar.copy(
                        out=ssb_flat[0:1, g * 1024 + h * 512:g * 1024 + (h + 1) * 512],
                        in_=pssb,
                    )
                nc.sync.dma_start(
                    out=ssb_scr[g * 1024:(g + 1) * 1024],
                    in_=ssb_flat[0:1, g * 1024:(g + 1) * 1024],
                )
                nc.sync.dma_start(
                    out=ssb[:, g * 8:(g + 1) * 8],
                    in_=ssb_scr[g * 1024:(g + 1) * 1024].rearrange(
                        "(m p) -> p m", p=128
                    ),
                )
                nc.scalar.activation(
                    out=lssb[:, g * 8:(g + 1) * 8], in_=ssb[:, g * 8:(g + 1) * 8],
                    func=AF.Ln,
                )
                nc.scalar.activation(
                    out=invb[:, g * 8:(g + 1) * 8], in_=lssb[:, g * 8:(g + 1) * 8],
                    func=AF.Exp, scale=-0.5,
                )

            # ---- S accumulator psum [1, 1024] ----
            S_ps = psacc.tile([1, MPC], F32, tag="S")

            # ---- main loop over 64 b-blocks ----
            for m in range(NB):
                ps = psim.tile([128, MPC], F32, tag="sim")
                for h in range(2):
                    nc.tensor.matmul(
                        ps[:, h * 512:(h + 1) * 512],
                        bT[:, m * 128:(m + 1) * 128],
                        aT[:, h * 512:(h + 1) * 512],
                        start=True, stop=True,
                    )
                if m < NA:
                    dscr = stage.tile([128, 128], F32, tag="dscr")
                    nc.vector.tensor_mul(
                        dscr, ps[:, m * 128:(m + 1) * 128], eye
                    )
                    nc.vector.tensor_reduce(
                        out=rdiag[:, m:m + 1], in_=dscr,
                        axis=mybir.AxisListType.X, op=OP.add,
                    )
                z = zpool.tile([128, MPC], F32, tag="z")
                nc.vector._custom_dve(
                    op, out=z, in0=ps, s0=invb[:, m:m + 1], s1=4.0
                )
                e = epool.tile([128, MPC], BF16, tag="e")
                nc.scalar.activation(out=e, in_=z, func=AF.Exp, bias=b_m16)
                if m < NA:
                    nc.gpsimd.tensor_mul(
                        e[:, m * 128:(m + 1) * 128],
                        e[:, m * 128:(m + 1) * 128],
                        antieye,
                    )
                for h in range(2):
                    nc.tensor.matmul(
                        S_ps[:, h * 512:(h + 1) * 512],
                        ones,
                        e[:, h * 512:(h + 1) * 512],
                        start=(m == 0), stop=(m == NB - 1),
                        skip_group_check=True,
                    )

            # ---- epilogue: per-row losses ----
            S_sb = stats.tile([1, MPC], F32, tag="S_sb")
            nc.scalar.copy(out=S_sb, in_=S_ps)
            Srs = stats.tile([128, NA], F32, tag="Srs")
            nc.sync.dma_start(out=S_scr[:], in_=S_sb[0:1, :])
            nc.sync.dma_start(
                out=Srs, in_=S_scr.rearrange("(m p) -> p m", p=128)
            )
            lse = stats.tile([128, NA], F32, tag="lse")
            nc.scalar.activation(out=lse, in_=Srs, func=AF.Ln)
            sdiag = stats.tile([128, NA], F32, tag="sdiag")
            nc.vector.tensor_mul(sdiag, rdiag, invb[:, 0:NA])
            w = stats.tile([128, NA], F32, tag="w")
            nc.vector.tensor_scalar(
                out=w, in0=sdiag, scalar1=12.0, scalar2=None, op0=OP.min
            )
            lpr = stats.tile([128, NA], F32, tag="lpr")
            nc.vector.scalar_tensor_tensor(
                out=lpr, in0=w, scalar=16.0, in1=w, op0=OP.subtract, op1=OP.mult
            )
            t = stats.tile([128, NA], F32, tag="t")
            nc.vector.scalar_tensor_tensor(
                out=t, in0=lpr, scalar=48.0, in1=lse, op0=OP.add, op1=OP.add
            )
            abst = stats.tile([128, NA], F32, tag="abst")
            nc.scalar.activation(out=abst, in_=t, func=AF.Abs)
            u = stats.tile([128, NA], F32, tag="u")
            nc.scalar.activation(out=u, in_=abst, func=AF.Exp, scale=-1.0)
            v = stats.tile([128, NA], F32, tag="v")
            nc.scalar.activation(out=v, in_=u, func=AF.Ln, bias=1.0)
            loss = stats.tile([128, NA], F32, tag="loss")
            nc.vector.scalar_tensor_tensor(
                out=loss, in0=t, scalar=0.0, in1=v, op0=OP.max, op1=OP.add
            )
            nc.sync.dma_start(out=out_pm, in_=loss)

    nc.finalize()
    _cache["nc"] = nc
    return nc


def kernel(embeddings_a: np.ndarray, embeddings_b: np.ndarray) -> np.ndarray:
    nc = _build()
    A = np.ascontiguousarray(embeddings_a, dtype=np.float32)
    Bm = np.ascontiguousarray(embeddings_b, dtype=np.float32)
    in_maps = []
    for c in range(NCORES):
        br = np.roll(Bm, -MPC * c, axis=0)
        in_maps.append(
            {
                "a_shard": A[MPC * c:MPC * (c + 1)],
                "bT": np.ascontiguousarray(br.T),
            }
        )
    res = run_bass_kernel_spmd(nc, in_maps, list(range(NCORES))).results
    losses = np.concatenate([res[c]["losses"] for c in range(NCORES)])
    return np.float32(np.mean(losses.astype(np.float64)))

